# revision 11
# baseline (speedup 1.0000x reference)
"""Trainium2 Bass kernel for the AttentionLSTM problem.

Tensor-parallel over the 4H gate dimension across 8 NeuronCores, with the
whole recurrence kept in TRANSPOSED layout (gate/h dim on partitions, batch
on the free axis).  Each core owns a 128-column slice of h; per timestep it
accumulates its gate pre-activations a^T = Wx^T x_t + Wh^T h_{t-1} in PSUM
as four [128, 64] blocks (i, f, o, g) of one bank, applies the LSTM
nonlinearity elementwise in the transposed layout, and produces its h^T
chunk [128, 64] directly -- no PE transpose and no PSUM->SBUF copy on the
critical path.  The chunk is DMA'd to DRAM and all-gathered (bf16, 16KB
payload, mesh algorithm ~5us) so every core has the full h^T for the next
step's matmuls.

The x@Wx pre-activations are computed LOOKAHEAD steps early in 2-step
groups (PE streams 128 x-columns per weight tile, halving LDWEIGHTS
overhead) while the AllGather is in flight; x spans are prefetched one
k-chunk per 2 steps to keep DMA/Q7 load smooth (burst refills skew the
per-step collectives).  The b_out->SBUF copies run split across the two
HWDGE queues (sync + scalar), and the Wh matmuls run j-outer so the first
chunks' matmuls overlap the second half's DMA.  Output is the bf16 h^T
chunk per step ([T, 128, N] per core); the host reassembles to (N, T, H)
fp32.

Measured on the 8-core trn2 harness: ~4.08-4.22 ms (vs 4.88 ms for the
batch-major v1 with PE transpose + PSUM copy + batched x refills), rel err
~3.9e-3 vs the fp32 reference.

Notes from this optimization session (see work/ for probes):
- remote_dma / remote_dma_broadcast (SBUF->SBUF cross-core with doorbell
  sems, which would replace the ~9.4us/step collective machinery with
  ~1-2us) CRASHES this axon runtime (NRT_EXEC_UNIT_UNRECOVERABLE) -- the
  instruction family appears unsupported by the deployed ucode/runtime.
- PSUM start=True clears has_written for the WHOLE BANK; use exactly one
  start per bank-tile accumulation group.
- The per-step critical path is: AllGather ~5.3 (incl ~1.1 trigger->begin
  and 2.6 peer-skew receive wait) + b_in DMA 0.6 + its ~1.4 HBM receipt +
  B DMAs 0.7 + ~1.8 completion receipt + wh MMs 1.9 (53ns/MM, LDW-bound)
  + gates 1.9.  G=2 batch ping-pong does NOT help (latency-bound chains).
"""

import os

import numpy as np

import concourse.bass as bass
import concourse.bacc as bacc
import concourse.mybir as mybir
from concourse import tile
from concourse.bass_utils import run_bass_kernel_spmd

F32 = mybir.dt.float32
BF16 = mybir.dt.bfloat16
AF = mybir.ActivationFunctionType


def _ensure_ntff_hook_module():
    """bass_utils imports antenv.axon_hooks for NTFF tracing under axon;
    this image's antenv lacks it.  Provide it, backed by the ctypes hook
    from trn_agent_boot when available (else tracing degrades to a no-op)."""
    import sys
    import types

    if "antenv.axon_hooks" in sys.modules:
        return
    try:
        import antenv.axon_hooks  # noqa: F401
        return
    except ImportError:
        pass
    hook = None
    try:
        from trn_agent_boot.trn_boot import _ntff_profile_via_ctypes
        hook = _ntff_profile_via_ctypes("/opt/axon/libaxon_pjrt.so")
    except Exception:
        hook = None
    mod = types.ModuleType("antenv.axon_hooks")
    mod._hook = hook
    mod.get_axon_ntff_profile_hook = lambda: mod._hook
    mod.set_axon_ntff_profile_hook = lambda h: setattr(mod, "_hook", h)
    sys.modules["antenv.axon_hooks"] = mod


_ensure_ntff_hook_module()

N, T, D, H = 64, 256, 1024, 1024
P = 128                 # SBUF partitions / PE tile
NCORES = 8
CH = H // NCORES        # 128 h-columns owned per core
GC = 4 * CH             # 512 gate columns per core
KT = D // P             # 8 contraction tiles
NQ = 4                  # gate blocks i,f,o,g
SPAN = 16               # timesteps of x per DMA span
LOOKAHEAD = int(os.environ.get("KERNEL_LA", "6"))
FILLER = int(os.environ.get("KERNEL_FILLER", "0"))

_cached = {}
last_result = None


def _build(with_bias: bool, n_steps: int = T):
    nc = bacc.Bacc("TRN2", target_bir_lowering=False, debug=False,
                   num_devices=NCORES)

    xT = nc.dram_tensor("xT", [D, T * N], BF16, kind="ExternalInput")
    wx = nc.dram_tensor("wx", [D, GC], BF16, kind="ExternalInput")
    wh = nc.dram_tensor("wh", [D, GC], BF16, kind="ExternalInput")
    ach = nc.dram_tensor("ach", [P, N * 100], F32, kind="ExternalInput")
    if with_bias:
        bvec = nc.dram_tensor("bvec", [1, GC], BF16, kind="ExternalInput")
        ones = nc.dram_tensor("ones", [1, N], BF16, kind="ExternalInput")
    outh = nc.dram_tensor("outh", [n_steps, P, N], BF16, kind="ExternalOutput")
    if FILLER:
        sinko = nc.dram_tensor("sinko", [P, 16], F32, kind="ExternalOutput")
    debug = os.environ.get("KERNEL_DEBUG", "0") == "1"
    if debug:
        dbga = nc.dram_tensor("dbga", [P, NQ, N], F32, kind="ExternalOutput")
        dbgb = nc.dram_tensor("dbgb", [P, KT, N], F32, kind="ExternalOutput")

    rg = [list(range(NCORES))]

    with tile.TileContext(nc) as tc:
        with (
            tc.tile_pool(name="const", bufs=1) as cpool,
            tc.tile_pool(name="x", bufs=2) as xpool,
            tc.tile_pool(name="work", bufs=2) as wpool,
            tc.tile_pool(name="hbuf", bufs=2) as hpool,
            tc.tile_pool(name="ps", bufs=4, space="PSUM") as pspool,
            tc.tile_pool(name="dram", bufs=2, space="DRAM") as dpool,
        ):
            # ---- weights ----
            wx_s = cpool.tile([P, KT, GC], BF16)
            wh_s = cpool.tile([P, KT, GC], BF16)
            for kt in range(KT):
                nc.sync.dma_start(out=wx_s[:, kt, :], in_=wx[kt * P:(kt + 1) * P, :])
                nc.sync.dma_start(out=wh_s[:, kt, :], in_=wh[kt * P:(kt + 1) * P, :])
            if with_bias:
                b_s = cpool.tile([1, GC], BF16)
                ones_s = cpool.tile([1, N], BF16)
                nc.sync.dma_start(out=b_s[:], in_=bvec[:])
                nc.sync.dma_start(out=ones_s[:], in_=ones[:])

            # ---- h0 = mean(A) for this core's 128 h-columns (transposed) ----
            a_s = cpool.tile([P, N * 100], F32)
            for q4 in range(4):
                nc.sync.dma_start(out=a_s[:, q4 * 1600:(q4 + 1) * 1600],
                                  in_=ach[:, q4 * 1600:(q4 + 1) * 1600])
            h0t = cpool.tile([P, N], F32)
            nc.vector.reduce_sum(h0t[:], a_s[:].rearrange("p (n q) -> p n q", q=100),
                                 axis=mybir.AxisListType.X)
            nc.scalar.activation(h0t[:], h0t[:], AF.Copy, bias=0.0, scale=0.01)
            # c0 = h0 chunk, already in [h-col, batch] layout
            c_prev = cpool.tile([P, N], F32)
            nc.vector.tensor_copy(c_prev[:], h0t[:])

            # initial exchange of h0^T
            h0b = wpool.tile([P, N], BF16, name="hs", tag="hs")
            nc.vector.tensor_copy(h0b[:], h0t[:])
            state = {"c": c_prev}

            def emit_exchange(h_tile):
                b_in = dpool.tile([P, N], BF16, name="b_in", tag="b_in")
                nc.sync.dma_start(out=b_in[:], in_=h_tile[:])
                b_out = dpool.tile([H, N], BF16, name="b_out", tag="b_out",
                                   addr_space="Shared")
                nc.gpsimd.collective_compute(
                    "AllGather", mybir.AluOpType.bypass, replica_groups=rg,
                    ins=[b_in[:]], outs=[b_out[:]])
                Bn = hpool.tile([P, KT, N], BF16, name="B", tag="B")
                hk = KT // 2
                for half, eng in ((0, nc.sync), (1, nc.scalar)):
                    eng.dma_start(
                        out=Bn[:, half * hk:(half + 1) * hk, :],
                        in_=b_out[half * hk * P:(half + 1) * hk * P, :]
                        .rearrange("(kt p) n -> p kt n", p=P))
                return Bn

            state["B"] = emit_exchange(h0b)

            ps_groups = {}
            xspan_s = None
            xspan_next = None
            fstate = {"first": True}
            if FILLER:
                ps_junk = pspool.tile([P, 512], F32, name="ps_junk",
                                      tag="ps_junk", bufs=1)

            def emit_filler():
                for _ in range(FILLER):
                    nc.tensor.matmul(ps_junk[:], lhsT=wh_s[:, 0, 0:P],
                                     rhs=wh_s[:, 1, :], start=fstate["first"],
                                     stop=False, skip_group_check=True)
                    fstate["first"] = False

            def emit_u_group(g):
                # pre-activations from x for steps 2g, 2g+1
                nonlocal xspan_s, xspan_next
                t0 = 2 * g
                if t0 == 0:
                    xspan_s = xpool.tile([P, KT, SPAN * N], BF16,
                                         name="xspan", tag="xspan")
                    for kt in range(KT):
                        nc.gpsimd.dma_start(
                            out=xspan_s[:, kt, :],
                            in_=xT[kt * P:(kt + 1) * P, 0:SPAN * N])
                elif t0 % SPAN == 0:
                    xspan_s, xspan_next = xspan_next, None
                # prefetch one chunk of the NEXT span per u-group emission
                sp_next = t0 // SPAN + 1
                kt = (t0 % SPAN) // 2
                if (sp_next + 1) * SPAN * N <= T * N and kt < KT:
                    if kt == 0:
                        xspan_next = xpool.tile([P, KT, SPAN * N], BF16,
                                                name="xspan", tag="xspan")
                    nc.gpsimd.dma_start(
                        out=xspan_next[:, kt, :],
                        in_=xT[kt * P:(kt + 1) * P,
                               sp_next * SPAN * N:(sp_next + 1) * SPAN * N])
                ps = pspool.tile([P, NQ, 2, N], F32, name="ps", tag="ps")
                ps_groups[g] = ps
                off = (t0 % SPAN) * N
                for q in range(NQ):
                    for j in range(KT):
                        nc.tensor.matmul(
                            ps[:, q, :, :],
                            lhsT=wx_s[:, j, q * CH:(q + 1) * CH],
                            rhs=xspan_s[:, j, off:off + 2 * N],
                            start=(q == 0 and j == 0), stop=False,
                            skip_group_check=True)
                    if with_bias:
                        # a^T[gate,n] += b[gate] * ones[n] (rank-1, K=1)
                        for hf in range(2):
                            nc.tensor.matmul(
                                ps[:, q, hf, :],
                                lhsT=b_s[:, q * CH:(q + 1) * CH],
                                rhs=ones_s[:],
                                start=False, stop=False, skip_group_check=True)

            def emit_step(t):
                g, hf = divmod(t, 2)
                ps = ps_groups[g]
                B = state["B"]
                # q-outer, ordered f,i,g,o: each gate's activation can start
                # as soon as ITS 8 matmuls finish, pipelining the nonlinear
                # chain under the remaining gates' matmuls
                gate = {}
                for q in (1, 0, 3, 2):
                    for j in range(KT):
                        nc.tensor.matmul(
                            ps[:, q, hf, :],
                            lhsT=wh_s[:, j, q * CH:(q + 1) * CH],
                            rhs=B[:, j, :],
                            start=False, stop=(j == KT - 1),
                            skip_group_check=True)
                    gt = wpool.tile([P, N], F32, name=f"g{q}", tag=f"g{q}")
                    nc.scalar.activation(
                        gt[:], ps[:, q, hf, :],
                        AF.Tanh if q == 3 else AF.Sigmoid)
                    gate[q] = gt
                    if q == 1:
                        c_new = wpool.tile([P, N], F32, name="c", tag="c")
                        nc.vector.tensor_mul(out=c_new[:], in0=gt[:],
                                             in1=state["c"][:])
                    elif q == 3:
                        ig = wpool.tile([P, N], F32, name="ig", tag="ig")
                        nc.vector.tensor_mul(out=ig[:], in0=gate[0][:],
                                             in1=gt[:])
                        nc.vector.tensor_add(out=c_new[:], in0=c_new[:],
                                             in1=ig[:])
                if hf == 1:
                    del ps_groups[g]
                state["c"] = c_new
                if debug and t == 0:
                    dt_ = cpool.tile([P, NQ, N], F32)
                    for q in range(NQ):
                        nc.vector.tensor_copy(dt_[:, q, :], ps[:, q, hf, :])
                    nc.sync.dma_start(out=dbga[:], in_=dt_[:])
                    dtb = cpool.tile([P, KT, N], F32)
                    nc.vector.tensor_copy(dtb[:], state["B"][:])
                    nc.sync.dma_start(out=dbgb[:], in_=dtb[:])
                tch = wpool.tile([P, N], F32, name="tch", tag="tch")
                nc.scalar.activation(tch[:], c_new[:], AF.Tanh)
                h_new = wpool.tile([P, N], BF16, name="hs", tag="hs")
                nc.vector.tensor_mul(out=h_new[:], in0=gate[2][:], in1=tch[:])
                nc.scalar.dma_start(out=outh[t], in_=h_new[:])
                if t < n_steps - 1:
                    state["B"] = emit_exchange(h_new)

            for ph in range(n_steps + LOOKAHEAD):
                if ph < n_steps and ph % 2 == 0:
                    emit_u_group(ph // 2)
                if FILLER:
                    emit_filler()
                t = ph - LOOKAHEAD
                if t >= 0:
                    emit_step(t)
            if FILLER:
                sink = cpool.tile([P, 16], F32)
                nc.vector.tensor_copy(sink[:], ps_junk[:, 0:16])
                nc.sync.dma_start(out=sinko[:], in_=sink[:])

    nc.compile()
    return nc


def kernel(x, A, Wx, Wh, b):
    import ml_dtypes

    x = np.ascontiguousarray(np.asarray(x, dtype=np.float32))
    A = np.ascontiguousarray(np.asarray(A, dtype=np.float32))
    Wx = np.asarray(Wx, dtype=np.float32)
    Wh = np.asarray(Wh, dtype=np.float32)
    b = np.asarray(b, dtype=np.float32)

    with_bias = bool(np.any(b))
    n_steps = int(os.environ.get("KERNEL_STEPS", T))
    key = (with_bias, n_steps)
    if key not in _cached:
        _cached[key] = _build(with_bias, n_steps)
    nc = _cached[key]

    mmnp = ml_dtypes.bfloat16
    xT_np = np.ascontiguousarray(
        x.transpose(2, 1, 0).reshape(D, T * N).astype(mmnp))

    in_maps = []
    for k in range(NCORES):
        cols = np.concatenate([np.arange(q * H + k * CH, q * H + k * CH + CH)
                               for q in range(4)])
        m = {
            "xT": xT_np,
            "wx": np.ascontiguousarray(Wx[:, cols].astype(mmnp)),
            "wh": np.ascontiguousarray(Wh[:, cols].astype(mmnp)),
            "ach": np.ascontiguousarray(
                A[:, k * CH:(k + 1) * CH].transpose(1, 0, 2, 3).reshape(P, N * 100)),
        }
        if with_bias:
            m["bvec"] = np.ascontiguousarray(b[cols].reshape(1, GC).astype(mmnp))
            m["ones"] = np.ones((1, N), dtype=mmnp)
        in_maps.append(m)

    res = run_bass_kernel_spmd(nc, in_maps, core_ids=list(range(NCORES)))
    global last_result
    last_result = res

    final = np.empty((N, n_steps, H), dtype=np.float32)
    for k in range(NCORES):
        # outh: [T, 128, N] bf16, h^T chunks -> final[n, t, k*CH + c]
        final[:, :, k * CH:(k + 1) * CH] = (
            res.results[k]["outh"].astype(np.float32).transpose(2, 0, 1))
    return final


# revision 12
# speedup vs baseline: 1.1287x; 1.1287x over previous
"""Trainium2 Bass kernel for the AttentionLSTM problem.

Tensor-parallel over the 4H gate dimension across 8 NeuronCores, with the
whole recurrence kept in TRANSPOSED layout (gate/h dim on partitions, batch
on the free axis).  Each core owns a 128-column slice of h; per timestep it
accumulates its gate pre-activations a^T = Wx^T x_t + Wh^T h_{t-1} in PSUM
as four [128, 64] blocks (i, f, o, g) of one bank, applies the LSTM
nonlinearity elementwise in the transposed layout, and produces its h^T
chunk [128, 64] directly -- no PE transpose and no PSUM->SBUF copy on the
critical path.  The chunk is DMA'd to DRAM and all-gathered (bf16, 16KB
payload, mesh algorithm ~5us) so every core has the full h^T for the next
step's matmuls.

The x@Wx pre-activations are computed LOOKAHEAD steps early in 2-step
groups (PE streams 128 x-columns per weight tile, halving LDWEIGHTS
overhead) while the AllGather is in flight; x spans are prefetched one
k-chunk per 2 steps to keep DMA/Q7 load smooth (burst refills skew the
per-step collectives).  The b_out->SBUF copies run split across the two
HWDGE queues (sync + scalar), and the Wh matmuls run j-outer so the first
chunks' matmuls overlap the second half's DMA.  Output is the bf16 h^T
chunk per step ([T, 128, N] per core); the host reassembles to (N, T, H)
fp32.

Measured on the 8-core trn2 harness: ~4.08-4.22 ms (vs 4.88 ms for the
batch-major v1 with PE transpose + PSUM copy + batched x refills), rel err
~3.9e-3 vs the fp32 reference.

Notes from this optimization session (see work/ for probes):
- remote_dma / remote_dma_broadcast (SBUF->SBUF cross-core with doorbell
  sems, which would replace the ~9.4us/step collective machinery with
  ~1-2us) CRASHES this axon runtime (NRT_EXEC_UNIT_UNRECOVERABLE) -- the
  instruction family appears unsupported by the deployed ucode/runtime.
- PSUM start=True clears has_written for the WHOLE BANK; use exactly one
  start per bank-tile accumulation group.
- The per-step critical path is: AllGather ~5.3 (incl ~1.1 trigger->begin
  and 2.6 peer-skew receive wait) + b_in DMA 0.6 + its ~1.4 HBM receipt +
  B DMAs 0.7 + ~1.8 completion receipt + wh MMs 1.9 (53ns/MM, LDW-bound)
  + gates 1.9.  G=2 batch ping-pong does NOT help (latency-bound chains).
"""

import os

import numpy as np

import concourse.bass as bass
import concourse.bacc as bacc
import concourse.mybir as mybir
from concourse import tile
from concourse.bass_utils import run_bass_kernel_spmd

F32 = mybir.dt.float32
BF16 = mybir.dt.bfloat16
AF = mybir.ActivationFunctionType


def _ensure_ntff_hook_module():
    """bass_utils imports antenv.axon_hooks for NTFF tracing under axon;
    this image's antenv lacks it.  Provide it, backed by the ctypes hook
    from trn_agent_boot when available (else tracing degrades to a no-op)."""
    import sys
    import types

    if "antenv.axon_hooks" in sys.modules:
        return
    try:
        import antenv.axon_hooks  # noqa: F401
        return
    except ImportError:
        pass
    hook = None
    try:
        from trn_agent_boot.trn_boot import _ntff_profile_via_ctypes
        hook = _ntff_profile_via_ctypes("/opt/axon/libaxon_pjrt.so")
    except Exception:
        hook = None
    mod = types.ModuleType("antenv.axon_hooks")
    mod._hook = hook
    mod.get_axon_ntff_profile_hook = lambda: mod._hook
    mod.set_axon_ntff_profile_hook = lambda h: setattr(mod, "_hook", h)
    sys.modules["antenv.axon_hooks"] = mod


_ensure_ntff_hook_module()

N, T, D, H = 64, 256, 1024, 1024
P = 128                 # SBUF partitions / PE tile
NCORES = 8
CH = H // NCORES        # 128 h-columns owned per core
GC = 4 * CH             # 512 gate columns per core
KT = D // P             # 8 contraction tiles
NQ = 4                  # gate blocks i,f,o,g
SPAN = 16               # timesteps of x per DMA span
LOOKAHEAD = int(os.environ.get("KERNEL_LA", "6"))
FILLER = int(os.environ.get("KERNEL_FILLER", "0"))

_cached = {}
last_result = None


def _build(with_bias: bool, n_steps: int = T):
    nc = bacc.Bacc("TRN2", target_bir_lowering=False, debug=False,
                   num_devices=NCORES)

    xT = nc.dram_tensor("xT", [D, T * N], BF16, kind="ExternalInput")
    wx = nc.dram_tensor("wx", [D, GC], BF16, kind="ExternalInput")
    wh = nc.dram_tensor("wh", [D, GC], BF16, kind="ExternalInput")
    ach = nc.dram_tensor("ach", [P, N * 100], F32, kind="ExternalInput")
    if with_bias:
        bvec = nc.dram_tensor("bvec", [1, GC], BF16, kind="ExternalInput")
        ones = nc.dram_tensor("ones", [1, N], BF16, kind="ExternalInput")
    outh = nc.dram_tensor("outh", [n_steps, P, N], BF16, kind="ExternalOutput")
    if FILLER:
        sinko = nc.dram_tensor("sinko", [P, 16], F32, kind="ExternalOutput")
    debug = os.environ.get("KERNEL_DEBUG", "0") == "1"
    if debug:
        dbga = nc.dram_tensor("dbga", [P, NQ, N], F32, kind="ExternalOutput")
        dbgb = nc.dram_tensor("dbgb", [P, KT, N], F32, kind="ExternalOutput")

    rg = [list(range(NCORES))]

    with tile.TileContext(nc) as tc:
        with (
            tc.tile_pool(name="const", bufs=1) as cpool,
            tc.tile_pool(name="x", bufs=2) as xpool,
            tc.tile_pool(name="work", bufs=2) as wpool,
            tc.tile_pool(name="hbuf", bufs=2) as hpool,
            tc.tile_pool(name="ps", bufs=4, space="PSUM") as pspool,
            tc.tile_pool(name="dram", bufs=2, space="DRAM") as dpool,
        ):
            # ---- weights ----
            wx_s = cpool.tile([P, KT, GC], BF16)
            wh_s = cpool.tile([P, KT, GC], BF16)
            for kt in range(KT):
                nc.sync.dma_start(out=wx_s[:, kt, :], in_=wx[kt * P:(kt + 1) * P, :])
                nc.sync.dma_start(out=wh_s[:, kt, :], in_=wh[kt * P:(kt + 1) * P, :])
            if with_bias:
                b_s = cpool.tile([1, GC], BF16)
                ones_s = cpool.tile([1, N], BF16)
                nc.sync.dma_start(out=b_s[:], in_=bvec[:])
                nc.sync.dma_start(out=ones_s[:], in_=ones[:])

            # ---- h0 = mean(A) for this core's 128 h-columns (transposed) ----
            a_s = cpool.tile([P, N * 100], F32)
            for q4 in range(4):
                nc.sync.dma_start(out=a_s[:, q4 * 1600:(q4 + 1) * 1600],
                                  in_=ach[:, q4 * 1600:(q4 + 1) * 1600])
            h0t = cpool.tile([P, N], F32)
            nc.vector.reduce_sum(h0t[:], a_s[:].rearrange("p (n q) -> p n q", q=100),
                                 axis=mybir.AxisListType.X)
            nc.scalar.activation(h0t[:], h0t[:], AF.Copy, bias=0.0, scale=0.01)
            # c0 = h0 chunk, already in [h-col, batch] layout
            c_prev = cpool.tile([P, N], F32)
            nc.vector.tensor_copy(c_prev[:], h0t[:])

            # initial exchange of h0^T
            h0b = wpool.tile([P, N], BF16, name="hs", tag="hs")
            nc.vector.tensor_copy(h0b[:], h0t[:])
            state = {"c": c_prev}

            def emit_exchange(h_tile):
                b_in = dpool.tile([P, N], BF16, name="b_in", tag="b_in")
                nc.sync.dma_start(out=b_in[:], in_=h_tile[:])
                b_out = dpool.tile([H, N], BF16, name="b_out", tag="b_out",
                                   addr_space="Shared")
                nc.gpsimd.collective_compute(
                    "AllGather", mybir.AluOpType.bypass, replica_groups=rg,
                    ins=[b_in[:]], outs=[b_out[:]])
                Bn = hpool.tile([P, KT, N], BF16, name="B", tag="B")
                hk = KT // 2
                for half, eng in ((0, nc.sync), (1, nc.scalar)):
                    eng.dma_start(
                        out=Bn[:, half * hk:(half + 1) * hk, :],
                        in_=b_out[half * hk * P:(half + 1) * hk * P, :]
                        .rearrange("(kt p) n -> p kt n", p=P))
                return Bn

            state["B"] = emit_exchange(h0b)

            ps_groups = {}
            xspan_s = None
            xspan_next = None
            fstate = {"first": True}
            if FILLER:
                ps_junk = pspool.tile([P, 512], F32, name="ps_junk",
                                      tag="ps_junk", bufs=1)

            def emit_filler():
                for _ in range(FILLER):
                    nc.tensor.matmul(ps_junk[:], lhsT=wh_s[:, 0, 0:P],
                                     rhs=wh_s[:, 1, :], start=fstate["first"],
                                     stop=False, skip_group_check=True)
                    fstate["first"] = False

            def emit_u_group(g):
                # pre-activations from x for steps 2g, 2g+1
                nonlocal xspan_s, xspan_next
                t0 = 2 * g
                if t0 == 0:
                    xspan_s = xpool.tile([P, KT, SPAN * N], BF16,
                                         name="xspan", tag="xspan")
                    for kt in range(KT):
                        nc.gpsimd.dma_start(
                            out=xspan_s[:, kt, :],
                            in_=xT[kt * P:(kt + 1) * P, 0:SPAN * N])
                elif t0 % SPAN == 0:
                    xspan_s, xspan_next = xspan_next, None
                # prefetch one chunk of the NEXT span per u-group emission
                sp_next = t0 // SPAN + 1
                kt = (t0 % SPAN) // 2
                if (sp_next + 1) * SPAN * N <= T * N and kt < KT:
                    if kt == 0:
                        xspan_next = xpool.tile([P, KT, SPAN * N], BF16,
                                                name="xspan", tag="xspan")
                    nc.gpsimd.dma_start(
                        out=xspan_next[:, kt, :],
                        in_=xT[kt * P:(kt + 1) * P,
                               sp_next * SPAN * N:(sp_next + 1) * SPAN * N])
                ps = pspool.tile([P, NQ, 2, N], F32, name="ps", tag="ps")
                ps_groups[g] = ps
                off = (t0 % SPAN) * N
                for q in range(NQ):
                    for j in range(KT):
                        nc.tensor.matmul(
                            ps[:, q, :, :],
                            lhsT=wx_s[:, j, q * CH:(q + 1) * CH],
                            rhs=xspan_s[:, j, off:off + 2 * N],
                            start=(q == 0 and j == 0), stop=False,
                            skip_group_check=True)
                    if with_bias:
                        # a^T[gate,n] += b[gate] * ones[n] (rank-1, K=1)
                        for hf in range(2):
                            nc.tensor.matmul(
                                ps[:, q, hf, :],
                                lhsT=b_s[:, q * CH:(q + 1) * CH],
                                rhs=ones_s[:],
                                start=False, stop=False, skip_group_check=True)

            def emit_step(t):
                g, hf = divmod(t, 2)
                ps = ps_groups[g]
                B = state["B"]
                for j in range(KT):
                    for q in range(NQ):
                        nc.tensor.matmul(
                            ps[:, q, hf, :],
                            lhsT=wh_s[:, j, q * CH:(q + 1) * CH],
                            rhs=B[:, j, :],
                            start=False, stop=(j == KT - 1),
                            skip_group_check=True)
                if hf == 1:
                    del ps_groups[g]
                if debug and t == 0:
                    dt_ = cpool.tile([P, NQ, N], F32)
                    for q in range(NQ):
                        nc.vector.tensor_copy(dt_[:, q, :], ps[:, q, hf, :])
                    nc.sync.dma_start(out=dbga[:], in_=dt_[:])
                    dtb = cpool.tile([P, KT, N], F32)
                    nc.vector.tensor_copy(dtb[:], state["B"][:])
                    nc.sync.dma_start(out=dbgb[:], in_=dtb[:])
                # gates (transposed layout): q blocks = i, f, o, g
                sig = wpool.tile([P, 3, N], F32, name="sig", tag="sig")
                nc.scalar.activation(sig[:], ps[:, 0:3, hf, :], AF.Sigmoid)
                gg = wpool.tile([P, N], F32, name="gg", tag="gg")
                nc.scalar.activation(gg[:], ps[:, 3, hf, :], AF.Tanh)
                c_new = wpool.tile([P, N], F32, name="c", tag="c")
                nc.vector.tensor_mul(out=c_new[:], in0=sig[:, 1, :],
                                     in1=state["c"][:])
                ig = wpool.tile([P, N], F32, name="ig", tag="ig")
                nc.vector.tensor_mul(out=ig[:], in0=sig[:, 0, :], in1=gg[:])
                nc.vector.tensor_add(out=c_new[:], in0=c_new[:], in1=ig[:])
                state["c"] = c_new
                tch = wpool.tile([P, N], F32, name="tch", tag="tch")
                nc.scalar.activation(tch[:], c_new[:], AF.Tanh)
                h_new = wpool.tile([P, N], BF16, name="hs", tag="hs")
                nc.vector.tensor_mul(out=h_new[:], in0=sig[:, 2, :], in1=tch[:])
                nc.scalar.dma_start(out=outh[t], in_=h_new[:])
                if t < n_steps - 1:
                    state["B"] = emit_exchange(h_new)

            for ph in range(n_steps + LOOKAHEAD):
                if ph < n_steps and ph % 2 == 0:
                    emit_u_group(ph // 2)
                if FILLER:
                    emit_filler()
                t = ph - LOOKAHEAD
                if t >= 0:
                    emit_step(t)
            if FILLER:
                sink = cpool.tile([P, 16], F32)
                nc.vector.tensor_copy(sink[:], ps_junk[:, 0:16])
                nc.sync.dma_start(out=sinko[:], in_=sink[:])

    nc.compile()
    return nc


def kernel(x, A, Wx, Wh, b):
    import ml_dtypes

    x = np.ascontiguousarray(np.asarray(x, dtype=np.float32))
    A = np.ascontiguousarray(np.asarray(A, dtype=np.float32))
    Wx = np.asarray(Wx, dtype=np.float32)
    Wh = np.asarray(Wh, dtype=np.float32)
    b = np.asarray(b, dtype=np.float32)

    with_bias = bool(np.any(b))
    n_steps = int(os.environ.get("KERNEL_STEPS", T))
    key = (with_bias, n_steps)
    if key not in _cached:
        _cached[key] = _build(with_bias, n_steps)
    nc = _cached[key]

    mmnp = ml_dtypes.bfloat16
    xT_np = np.ascontiguousarray(
        x.transpose(2, 1, 0).reshape(D, T * N).astype(mmnp))

    in_maps = []
    for k in range(NCORES):
        cols = np.concatenate([np.arange(q * H + k * CH, q * H + k * CH + CH)
                               for q in range(4)])
        m = {
            "xT": xT_np,
            "wx": np.ascontiguousarray(Wx[:, cols].astype(mmnp)),
            "wh": np.ascontiguousarray(Wh[:, cols].astype(mmnp)),
            "ach": np.ascontiguousarray(
                A[:, k * CH:(k + 1) * CH].transpose(1, 0, 2, 3).reshape(P, N * 100)),
        }
        if with_bias:
            m["bvec"] = np.ascontiguousarray(b[cols].reshape(1, GC).astype(mmnp))
            m["ones"] = np.ones((1, N), dtype=mmnp)
        in_maps.append(m)

    res = run_bass_kernel_spmd(nc, in_maps, core_ids=list(range(NCORES)))
    global last_result
    last_result = res

    final = np.empty((N, n_steps, H), dtype=np.float32)
    for k in range(NCORES):
        # outh: [T, 128, N] bf16, h^T chunks -> final[n, t, k*CH + c]
        final[:, :, k * CH:(k + 1) * CH] = (
            res.results[k]["outh"].astype(np.float32).transpose(2, 0, 1))
    return final


# revision 13
# speedup vs baseline: 1.1479x; 1.0170x over previous
"""Trainium2 Bass kernel for the AttentionLSTM problem.

Tensor-parallel over the 4H gate dimension across 8 NeuronCores, with the
whole recurrence kept in TRANSPOSED layout (gate/h dim on partitions, batch
on the free axis).  Each core owns a 128-column slice of h; per timestep it
accumulates its gate pre-activations a^T = Wx^T x_t + Wh^T h_{t-1} in PSUM
as four [128, 64] blocks (i, f, o, g) of one bank, applies the LSTM
nonlinearity elementwise in the transposed layout, and produces its h^T
chunk [128, 64] directly -- no PE transpose and no PSUM->SBUF copy on the
critical path.  The chunk is DMA'd to DRAM and all-gathered (bf16, 16KB
payload, mesh algorithm ~5us) so every core has the full h^T for the next
step's matmuls.

The x@Wx pre-activations are computed LOOKAHEAD steps early in 2-step
groups (PE streams 128 x-columns per weight tile, halving LDWEIGHTS
overhead) while the AllGather is in flight; x spans are prefetched one
k-chunk per 2 steps to keep DMA/Q7 load smooth (burst refills skew the
per-step collectives).  The b_out->SBUF copies run split across the two
HWDGE queues (sync + scalar), and the Wh matmuls run j-outer so the first
chunks' matmuls overlap the second half's DMA.  Output is the bf16 h^T
chunk per step ([T, 128, N] per core); the host reassembles to (N, T, H)
fp32.

Measured on the 8-core trn2 harness: ~4.08-4.22 ms (vs 4.88 ms for the
batch-major v1 with PE transpose + PSUM copy + batched x refills), rel err
~3.9e-3 vs the fp32 reference.

Notes from this optimization session (see work/ for probes):
- remote_dma / remote_dma_broadcast (SBUF->SBUF cross-core with doorbell
  sems, which would replace the ~9.4us/step collective machinery with
  ~1-2us) CRASHES this axon runtime (NRT_EXEC_UNIT_UNRECOVERABLE) -- the
  instruction family appears unsupported by the deployed ucode/runtime.
- PSUM start=True clears has_written for the WHOLE BANK; use exactly one
  start per bank-tile accumulation group.
- The per-step critical path is: AllGather ~5.3 (incl ~1.1 trigger->begin
  and 2.6 peer-skew receive wait) + b_in DMA 0.6 + its ~1.4 HBM receipt +
  B DMAs 0.7 + ~1.8 completion receipt + wh MMs 1.9 (53ns/MM, LDW-bound)
  + gates 1.9.  G=2 batch ping-pong does NOT help (latency-bound chains).
"""

import os

import numpy as np

import concourse.bass as bass
import concourse.bacc as bacc
import concourse.mybir as mybir
from concourse import tile
from concourse.bass_utils import run_bass_kernel_spmd

F32 = mybir.dt.float32
BF16 = mybir.dt.bfloat16
AF = mybir.ActivationFunctionType


def _ensure_ntff_hook_module():
    """bass_utils imports antenv.axon_hooks for NTFF tracing under axon;
    this image's antenv lacks it.  Provide it, backed by the ctypes hook
    from trn_agent_boot when available (else tracing degrades to a no-op)."""
    import sys
    import types

    if "antenv.axon_hooks" in sys.modules:
        return
    try:
        import antenv.axon_hooks  # noqa: F401
        return
    except ImportError:
        pass
    hook = None
    try:
        from trn_agent_boot.trn_boot import _ntff_profile_via_ctypes
        hook = _ntff_profile_via_ctypes("/opt/axon/libaxon_pjrt.so")
    except Exception:
        hook = None
    mod = types.ModuleType("antenv.axon_hooks")
    mod._hook = hook
    mod.get_axon_ntff_profile_hook = lambda: mod._hook
    mod.set_axon_ntff_profile_hook = lambda h: setattr(mod, "_hook", h)
    sys.modules["antenv.axon_hooks"] = mod


_ensure_ntff_hook_module()

N, T, D, H = 64, 256, 1024, 1024
P = 128                 # SBUF partitions / PE tile
NCORES = 8
CH = H // NCORES        # 128 h-columns owned per core
GC = 4 * CH             # 512 gate columns per core
KT = D // P             # 8 contraction tiles
NQ = 4                  # gate blocks i,f,o,g
SPAN = 16               # timesteps of x per DMA span
LOOKAHEAD = int(os.environ.get("KERNEL_LA", "6"))
FILLER = int(os.environ.get("KERNEL_FILLER", "0"))

_cached = {}
last_result = None


def _build(with_bias: bool, n_steps: int = T):
    nc = bacc.Bacc("TRN2", target_bir_lowering=False, debug=False,
                   num_devices=NCORES)

    xT = nc.dram_tensor("xT", [D, T * N], BF16, kind="ExternalInput")
    wx = nc.dram_tensor("wx", [D, GC], BF16, kind="ExternalInput")
    wh = nc.dram_tensor("wh", [D, GC], BF16, kind="ExternalInput")
    ach = nc.dram_tensor("ach", [P, N * 100], F32, kind="ExternalInput")
    if with_bias:
        bvec = nc.dram_tensor("bvec", [1, GC], BF16, kind="ExternalInput")
        ones = nc.dram_tensor("ones", [1, N], BF16, kind="ExternalInput")
    outh = nc.dram_tensor("outh", [n_steps, P, N], BF16, kind="ExternalOutput")
    if FILLER:
        sinko = nc.dram_tensor("sinko", [P, 16], F32, kind="ExternalOutput")
    debug = os.environ.get("KERNEL_DEBUG", "0") == "1"
    if debug:
        dbga = nc.dram_tensor("dbga", [P, NQ, N], F32, kind="ExternalOutput")
        dbgb = nc.dram_tensor("dbgb", [P, KT, N], F32, kind="ExternalOutput")

    rg = [list(range(NCORES))]

    with tile.TileContext(nc) as tc:
        with (
            tc.tile_pool(name="const", bufs=1) as cpool,
            tc.tile_pool(name="x", bufs=2) as xpool,
            tc.tile_pool(name="work", bufs=2) as wpool,
            tc.tile_pool(name="hbuf", bufs=2) as hpool,
            tc.tile_pool(name="ps", bufs=4, space="PSUM") as pspool,
            tc.tile_pool(name="dram", bufs=2, space="DRAM") as dpool,
        ):
            # ---- weights ----
            wx_s = cpool.tile([P, KT, GC], BF16)
            wh_s = cpool.tile([P, KT, GC], BF16)
            for kt in range(KT):
                nc.sync.dma_start(out=wx_s[:, kt, :], in_=wx[kt * P:(kt + 1) * P, :])
                nc.sync.dma_start(out=wh_s[:, kt, :], in_=wh[kt * P:(kt + 1) * P, :])
            if with_bias:
                b_s = cpool.tile([1, GC], BF16)
                ones_s = cpool.tile([1, N], BF16)
                nc.sync.dma_start(out=b_s[:], in_=bvec[:])
                nc.sync.dma_start(out=ones_s[:], in_=ones[:])

            # ---- h0 = mean(A) for this core's 128 h-columns (transposed) ----
            a_s = cpool.tile([P, N * 100], F32)
            for q4 in range(4):
                nc.sync.dma_start(out=a_s[:, q4 * 1600:(q4 + 1) * 1600],
                                  in_=ach[:, q4 * 1600:(q4 + 1) * 1600])
            h0t = cpool.tile([P, N], F32)
            nc.vector.reduce_sum(h0t[:], a_s[:].rearrange("p (n q) -> p n q", q=100),
                                 axis=mybir.AxisListType.X)
            nc.scalar.activation(h0t[:], h0t[:], AF.Copy, bias=0.0, scale=0.01)
            # c0 = h0 chunk, already in [h-col, batch] layout
            c_prev = cpool.tile([P, N], F32)
            nc.vector.tensor_copy(c_prev[:], h0t[:])

            # initial exchange of h0^T
            h0b = wpool.tile([P, N], BF16, name="hs", tag="hs")
            nc.vector.tensor_copy(h0b[:], h0t[:])
            state = {"c": c_prev}

            def emit_exchange(h_tile):
                b_in = dpool.tile([P, N], BF16, name="b_in", tag="b_in")
                nc.sync.dma_start(out=b_in[:], in_=h_tile[:])
                b_out = dpool.tile([H, N], BF16, name="b_out", tag="b_out",
                                   addr_space="Shared")
                nc.gpsimd.collective_compute(
                    "AllGather", mybir.AluOpType.bypass, replica_groups=rg,
                    ins=[b_in[:]], outs=[b_out[:]])
                Bn = hpool.tile([P, KT, N], BF16, name="B", tag="B")
                hk = KT // 2
                for half, eng in ((0, nc.sync), (1, nc.scalar)):
                    eng.dma_start(
                        out=Bn[:, half * hk:(half + 1) * hk, :],
                        in_=b_out[half * hk * P:(half + 1) * hk * P, :]
                        .rearrange("(kt p) n -> p kt n", p=P))
                return Bn

            state["B"] = emit_exchange(h0b)

            ps_groups = {}
            xspan_s = None
            xspan_next = None
            fstate = {"first": True}
            if FILLER:
                ps_junk = pspool.tile([P, 512], F32, name="ps_junk",
                                      tag="ps_junk", bufs=1)

            def emit_filler():
                for _ in range(FILLER):
                    nc.tensor.matmul(ps_junk[:], lhsT=wh_s[:, 0, 0:P],
                                     rhs=wh_s[:, 1, :], start=fstate["first"],
                                     stop=False, skip_group_check=True)
                    fstate["first"] = False

            def emit_u_group(g):
                # pre-activations from x for steps 2g, 2g+1
                nonlocal xspan_s, xspan_next
                t0 = 2 * g
                if t0 == 0:
                    xspan_s = xpool.tile([P, KT, SPAN * N], BF16,
                                         name="xspan", tag="xspan")
                    for kt in range(KT):
                        nc.gpsimd.dma_start(
                            out=xspan_s[:, kt, :],
                            in_=xT[kt * P:(kt + 1) * P, 0:SPAN * N])
                elif t0 % SPAN == 0:
                    xspan_s, xspan_next = xspan_next, None
                # prefetch one chunk of the NEXT span per u-group emission
                sp_next = t0 // SPAN + 1
                kt = (t0 % SPAN) // 2
                if (sp_next + 1) * SPAN * N <= T * N and kt < KT:
                    if kt == 0:
                        xspan_next = xpool.tile([P, KT, SPAN * N], BF16,
                                                name="xspan", tag="xspan")
                    nc.gpsimd.dma_start(
                        out=xspan_next[:, kt, :],
                        in_=xT[kt * P:(kt + 1) * P,
                               sp_next * SPAN * N:(sp_next + 1) * SPAN * N])
                # two PSUM banks per group: A = gates {f,i}, B = {g,o};
                # lets the f/i sigmoid (bank-A read) overlap the g/o wh
                # matmuls (bank-B writes) without a same-bank PE-W/ACT-R
                # hazard
                psA = pspool.tile([P, 2, 2, N], F32, name="psA", tag="psA")
                psB = pspool.tile([P, 2, 2, N], F32, name="psB", tag="psB")
                ps_groups[g] = (psA, psB)
                off = (t0 % SPAN) * N
                for pst, qs in ((psA, (1, 0)), (psB, (3, 2))):
                    for qi, q in enumerate(qs):
                        for j in range(KT):
                            nc.tensor.matmul(
                                pst[:, qi, :, :],
                                lhsT=wx_s[:, j, q * CH:(q + 1) * CH],
                                rhs=xspan_s[:, j, off:off + 2 * N],
                                start=(qi == 0 and j == 0), stop=False,
                                skip_group_check=True)
                        if with_bias:
                            for hf in range(2):
                                nc.tensor.matmul(
                                    pst[:, qi, hf, :],
                                    lhsT=b_s[:, q * CH:(q + 1) * CH],
                                    rhs=ones_s[:],
                                    start=False, stop=False,
                                    skip_group_check=True)

            def emit_step(t):
                g, hf = divmod(t, 2)
                psA, psB = ps_groups[g]
                B = state["B"]
                for j in range(KT):
                    for qi, q in ((0, 1), (1, 0)):
                        nc.tensor.matmul(
                            psA[:, qi, hf, :],
                            lhsT=wh_s[:, j, q * CH:(q + 1) * CH],
                            rhs=B[:, j, :],
                            start=False, stop=(j == KT - 1),
                            skip_group_check=True)
                if debug and t == 0:
                    dt_ = cpool.tile([P, NQ, N], F32)
                    for q in range(NQ):
                        nc.vector.tensor_copy(dt_[:, q, :], ps[:, q, hf, :])
                    nc.sync.dma_start(out=dbga[:], in_=dt_[:])
                    dtb = cpool.tile([P, KT, N], F32)
                    nc.vector.tensor_copy(dtb[:], state["B"][:])
                    nc.sync.dma_start(out=dbgb[:], in_=dtb[:])
                # f,i sigmoid on bank A; overlaps the g,o matmuls below
                sig = wpool.tile([P, 2, N], F32, name="sig", tag="sig")
                nc.scalar.activation(sig[:], psA[:, :, hf, :], AF.Sigmoid)
                c_new = wpool.tile([P, N], F32, name="c", tag="c")
                nc.vector.tensor_mul(out=c_new[:], in0=sig[:, 0, :],
                                     in1=state["c"][:])
                for j in range(KT):
                    for qi, q in ((0, 3), (1, 2)):
                        nc.tensor.matmul(
                            psB[:, qi, hf, :],
                            lhsT=wh_s[:, j, q * CH:(q + 1) * CH],
                            rhs=B[:, j, :],
                            start=False, stop=(j == KT - 1),
                            skip_group_check=True)
                if hf == 1:
                    del ps_groups[g]
                gg = wpool.tile([P, N], F32, name="gg", tag="gg")
                nc.scalar.activation(gg[:], psB[:, 0, hf, :], AF.Tanh)
                so = wpool.tile([P, N], F32, name="so", tag="so")
                nc.scalar.activation(so[:], psB[:, 1, hf, :], AF.Sigmoid)
                ig = wpool.tile([P, N], F32, name="ig", tag="ig")
                nc.vector.tensor_mul(out=ig[:], in0=sig[:, 1, :], in1=gg[:])
                nc.vector.tensor_add(out=c_new[:], in0=c_new[:], in1=ig[:])
                state["c"] = c_new
                tch = wpool.tile([P, N], F32, name="tch", tag="tch")
                nc.scalar.activation(tch[:], c_new[:], AF.Tanh)
                h_new = wpool.tile([P, N], BF16, name="hs", tag="hs")
                nc.vector.tensor_mul(out=h_new[:], in0=so[:], in1=tch[:])
                nc.scalar.dma_start(out=outh[t], in_=h_new[:])
                if t < n_steps - 1:
                    state["B"] = emit_exchange(h_new)

            for ph in range(n_steps + LOOKAHEAD):
                if ph < n_steps and ph % 2 == 0:
                    emit_u_group(ph // 2)
                if FILLER:
                    emit_filler()
                t = ph - LOOKAHEAD
                if t >= 0:
                    emit_step(t)
            if FILLER:
                sink = cpool.tile([P, 16], F32)
                nc.vector.tensor_copy(sink[:], ps_junk[:, 0:16])
                nc.sync.dma_start(out=sinko[:], in_=sink[:])

    nc.compile()
    return nc


def kernel(x, A, Wx, Wh, b):
    import ml_dtypes

    x = np.ascontiguousarray(np.asarray(x, dtype=np.float32))
    A = np.ascontiguousarray(np.asarray(A, dtype=np.float32))
    Wx = np.asarray(Wx, dtype=np.float32)
    Wh = np.asarray(Wh, dtype=np.float32)
    b = np.asarray(b, dtype=np.float32)

    with_bias = bool(np.any(b))
    n_steps = int(os.environ.get("KERNEL_STEPS", T))
    key = (with_bias, n_steps)
    if key not in _cached:
        _cached[key] = _build(with_bias, n_steps)
    nc = _cached[key]

    mmnp = ml_dtypes.bfloat16
    xT_np = np.ascontiguousarray(
        x.transpose(2, 1, 0).reshape(D, T * N).astype(mmnp))

    in_maps = []
    for k in range(NCORES):
        cols = np.concatenate([np.arange(q * H + k * CH, q * H + k * CH + CH)
                               for q in range(4)])
        m = {
            "xT": xT_np,
            "wx": np.ascontiguousarray(Wx[:, cols].astype(mmnp)),
            "wh": np.ascontiguousarray(Wh[:, cols].astype(mmnp)),
            "ach": np.ascontiguousarray(
                A[:, k * CH:(k + 1) * CH].transpose(1, 0, 2, 3).reshape(P, N * 100)),
        }
        if with_bias:
            m["bvec"] = np.ascontiguousarray(b[cols].reshape(1, GC).astype(mmnp))
            m["ones"] = np.ones((1, N), dtype=mmnp)
        in_maps.append(m)

    res = run_bass_kernel_spmd(nc, in_maps, core_ids=list(range(NCORES)))
    global last_result
    last_result = res

    final = np.empty((N, n_steps, H), dtype=np.float32)
    for k in range(NCORES):
        # outh: [T, 128, N] bf16, h^T chunks -> final[n, t, k*CH + c]
        final[:, :, k * CH:(k + 1) * CH] = (
            res.results[k]["outh"].astype(np.float32).transpose(2, 0, 1))
    return final


# revision 14
# speedup vs baseline: 1.1550x; 1.0062x over previous
"""Trainium2 Bass kernel for the AttentionLSTM problem.

Tensor-parallel over the 4H gate dimension across 8 NeuronCores, with the
whole recurrence kept in TRANSPOSED layout (gate/h dim on partitions, batch
on the free axis).  Each core owns a 128-column slice of h; per timestep it
accumulates its gate pre-activations a^T = Wx^T x_t + Wh^T h_{t-1} in PSUM
as four [128, 64] blocks (i, f, o, g) of one bank, applies the LSTM
nonlinearity elementwise in the transposed layout, and produces its h^T
chunk [128, 64] directly -- no PE transpose and no PSUM->SBUF copy on the
critical path.  The chunk is DMA'd to DRAM and all-gathered (bf16, 16KB
payload, mesh algorithm ~5us) so every core has the full h^T for the next
step's matmuls.

The x@Wx pre-activations are computed LOOKAHEAD steps early in 2-step
groups (PE streams 128 x-columns per weight tile, halving LDWEIGHTS
overhead) while the AllGather is in flight; x spans are prefetched one
k-chunk per 2 steps to keep DMA/Q7 load smooth (burst refills skew the
per-step collectives).  The b_out->SBUF copies run split across the two
HWDGE queues (sync + scalar), and the Wh matmuls run j-outer so the first
chunks' matmuls overlap the second half's DMA.  Output is the bf16 h^T
chunk per step ([T, 128, N] per core); the host reassembles to (N, T, H)
fp32.

Measured on the 8-core trn2 harness: ~4.05-4.10 ms (two-bank gate split)
(previously ~4.08-4.22 ms single-bank; (vs 4.88 ms for the
batch-major v1 with PE transpose + PSUM copy + batched x refills), rel err
~3.9e-3 vs the fp32 reference.

Notes from this optimization session (see work/ for probes):
- remote_dma / remote_dma_broadcast (SBUF->SBUF cross-core with doorbell
  sems, which would replace the ~9.4us/step collective machinery with
  ~1-2us) CRASHES this axon runtime (NRT_EXEC_UNIT_UNRECOVERABLE) -- the
  instruction family appears unsupported by the deployed ucode/runtime.
- PSUM start=True clears has_written for the WHOLE BANK; use exactly one
  start per bank-tile accumulation group.
- The per-step critical path is: AllGather ~5.3 (incl ~1.1 trigger->begin
  and 2.6 peer-skew receive wait) + b_in DMA 0.6 + its ~1.4 HBM receipt +
  B DMAs 0.7 + ~1.8 completion receipt + wh MMs 1.9 (53ns/MM, LDW-bound)
  + gates 1.9.  G=2 batch ping-pong does NOT help (latency-bound chains).
"""

import os

import numpy as np

import concourse.bass as bass
import concourse.bacc as bacc
import concourse.mybir as mybir
from concourse import tile
from concourse.bass_utils import run_bass_kernel_spmd

F32 = mybir.dt.float32
BF16 = mybir.dt.bfloat16
AF = mybir.ActivationFunctionType


def _ensure_ntff_hook_module():
    """bass_utils imports antenv.axon_hooks for NTFF tracing under axon;
    this image's antenv lacks it.  Provide it, backed by the ctypes hook
    from trn_agent_boot when available (else tracing degrades to a no-op)."""
    import sys
    import types

    if "antenv.axon_hooks" in sys.modules:
        return
    try:
        import antenv.axon_hooks  # noqa: F401
        return
    except ImportError:
        pass
    hook = None
    try:
        from trn_agent_boot.trn_boot import _ntff_profile_via_ctypes
        hook = _ntff_profile_via_ctypes("/opt/axon/libaxon_pjrt.so")
    except Exception:
        hook = None
    mod = types.ModuleType("antenv.axon_hooks")
    mod._hook = hook
    mod.get_axon_ntff_profile_hook = lambda: mod._hook
    mod.set_axon_ntff_profile_hook = lambda h: setattr(mod, "_hook", h)
    sys.modules["antenv.axon_hooks"] = mod


_ensure_ntff_hook_module()

N, T, D, H = 64, 256, 1024, 1024
P = 128                 # SBUF partitions / PE tile
NCORES = 8
CH = H // NCORES        # 128 h-columns owned per core
GC = 4 * CH             # 512 gate columns per core
KT = D // P             # 8 contraction tiles
NQ = 4                  # gate blocks i,f,o,g
SPAN = 16               # timesteps of x per DMA span
LOOKAHEAD = int(os.environ.get("KERNEL_LA", "6"))
FILLER = int(os.environ.get("KERNEL_FILLER", "0"))

_cached = {}
last_result = None


def _build(with_bias: bool, n_steps: int = T):
    nc = bacc.Bacc("TRN2", target_bir_lowering=False, debug=False,
                   num_devices=NCORES)

    xT = nc.dram_tensor("xT", [D, T * N], BF16, kind="ExternalInput")
    wx = nc.dram_tensor("wx", [D, GC], BF16, kind="ExternalInput")
    wh = nc.dram_tensor("wh", [D, GC], BF16, kind="ExternalInput")
    ach = nc.dram_tensor("ach", [P, N * 100], F32, kind="ExternalInput")
    if with_bias:
        bvec = nc.dram_tensor("bvec", [1, GC], BF16, kind="ExternalInput")
        ones = nc.dram_tensor("ones", [1, N], BF16, kind="ExternalInput")
    outh = nc.dram_tensor("outh", [n_steps, P, N], BF16, kind="ExternalOutput")
    if FILLER:
        sinko = nc.dram_tensor("sinko", [P, 16], F32, kind="ExternalOutput")
    debug = os.environ.get("KERNEL_DEBUG", "0") == "1"
    if debug:
        dbga = nc.dram_tensor("dbga", [P, NQ, N], F32, kind="ExternalOutput")
        dbgb = nc.dram_tensor("dbgb", [P, KT, N], F32, kind="ExternalOutput")

    rg = [list(range(NCORES))]

    with tile.TileContext(nc) as tc:
        with (
            tc.tile_pool(name="const", bufs=1) as cpool,
            tc.tile_pool(name="x", bufs=2) as xpool,
            tc.tile_pool(name="work", bufs=2) as wpool,
            tc.tile_pool(name="hbuf", bufs=2) as hpool,
            tc.tile_pool(name="ps", bufs=4, space="PSUM") as pspool,
            tc.tile_pool(name="dram", bufs=2, space="DRAM") as dpool,
        ):
            # ---- weights ----
            wx_s = cpool.tile([P, KT, GC], BF16)
            wh_s = cpool.tile([P, KT, GC], BF16)
            for kt in range(KT):
                nc.sync.dma_start(out=wx_s[:, kt, :], in_=wx[kt * P:(kt + 1) * P, :])
                nc.sync.dma_start(out=wh_s[:, kt, :], in_=wh[kt * P:(kt + 1) * P, :])
            if with_bias:
                b_s = cpool.tile([1, GC], BF16)
                ones_s = cpool.tile([1, N], BF16)
                nc.sync.dma_start(out=b_s[:], in_=bvec[:])
                nc.sync.dma_start(out=ones_s[:], in_=ones[:])

            # ---- h0 = mean(A) for this core's 128 h-columns (transposed) ----
            a_s = cpool.tile([P, N * 100], F32)
            for q4 in range(4):
                nc.sync.dma_start(out=a_s[:, q4 * 1600:(q4 + 1) * 1600],
                                  in_=ach[:, q4 * 1600:(q4 + 1) * 1600])
            h0t = cpool.tile([P, N], F32)
            nc.vector.reduce_sum(h0t[:], a_s[:].rearrange("p (n q) -> p n q", q=100),
                                 axis=mybir.AxisListType.X)
            nc.scalar.activation(h0t[:], h0t[:], AF.Copy, bias=0.0, scale=0.01)
            # c0 = h0 chunk, already in [h-col, batch] layout
            c_prev = cpool.tile([P, N], F32)
            nc.vector.tensor_copy(c_prev[:], h0t[:])

            # initial exchange of h0^T
            h0b = wpool.tile([P, N], BF16, name="hs", tag="hs")
            nc.vector.tensor_copy(h0b[:], h0t[:])
            state = {"c": c_prev}

            def emit_exchange(h_tile):
                b_in = dpool.tile([P, N], BF16, name="b_in", tag="b_in")
                nc.sync.dma_start(out=b_in[:], in_=h_tile[:])
                b_out = dpool.tile([H, N], BF16, name="b_out", tag="b_out",
                                   addr_space="Shared")
                nc.gpsimd.collective_compute(
                    "AllGather", mybir.AluOpType.bypass, replica_groups=rg,
                    ins=[b_in[:]], outs=[b_out[:]])
                Bn = hpool.tile([P, KT, N], BF16, name="B", tag="B")
                hk = KT // 2
                for half, eng in ((0, nc.sync), (1, nc.scalar)):
                    eng.dma_start(
                        out=Bn[:, half * hk:(half + 1) * hk, :],
                        in_=b_out[half * hk * P:(half + 1) * hk * P, :]
                        .rearrange("(kt p) n -> p kt n", p=P))
                return Bn

            state["B"] = emit_exchange(h0b)

            ps_groups = {}
            xspan_s = None
            xspan_next = None
            fstate = {"first": True}
            if FILLER:
                ps_junk = pspool.tile([P, 512], F32, name="ps_junk",
                                      tag="ps_junk", bufs=1)

            def emit_filler():
                for _ in range(FILLER):
                    nc.tensor.matmul(ps_junk[:], lhsT=wh_s[:, 0, 0:P],
                                     rhs=wh_s[:, 1, :], start=fstate["first"],
                                     stop=False, skip_group_check=True)
                    fstate["first"] = False

            def emit_u_group(g):
                # pre-activations from x for steps 2g, 2g+1
                nonlocal xspan_s, xspan_next
                t0 = 2 * g
                if t0 == 0:
                    xspan_s = xpool.tile([P, KT, SPAN * N], BF16,
                                         name="xspan", tag="xspan")
                    for kt in range(KT):
                        nc.gpsimd.dma_start(
                            out=xspan_s[:, kt, :],
                            in_=xT[kt * P:(kt + 1) * P, 0:SPAN * N])
                elif t0 % SPAN == 0:
                    xspan_s, xspan_next = xspan_next, None
                # prefetch one chunk of the NEXT span per u-group emission
                sp_next = t0 // SPAN + 1
                kt = (t0 % SPAN) // 2
                if (sp_next + 1) * SPAN * N <= T * N and kt < KT:
                    if kt == 0:
                        xspan_next = xpool.tile([P, KT, SPAN * N], BF16,
                                                name="xspan", tag="xspan")
                    nc.gpsimd.dma_start(
                        out=xspan_next[:, kt, :],
                        in_=xT[kt * P:(kt + 1) * P,
                               sp_next * SPAN * N:(sp_next + 1) * SPAN * N])
                # two PSUM banks per group: A = gates {f,i}, B = {g,o};
                # lets the f/i sigmoid (bank-A read) overlap the g/o wh
                # matmuls (bank-B writes) without a same-bank PE-W/ACT-R
                # hazard
                psA = pspool.tile([P, 2, 2, N], F32, name="psA", tag="psA")
                psB = pspool.tile([P, 2, 2, N], F32, name="psB", tag="psB")
                ps_groups[g] = (psA, psB)
                off = (t0 % SPAN) * N
                for pst, qs in ((psA, (1, 0)), (psB, (3, 2))):
                    for qi, q in enumerate(qs):
                        for j in range(KT):
                            nc.tensor.matmul(
                                pst[:, qi, :, :],
                                lhsT=wx_s[:, j, q * CH:(q + 1) * CH],
                                rhs=xspan_s[:, j, off:off + 2 * N],
                                start=(qi == 0 and j == 0), stop=False,
                                skip_group_check=True)
                        if with_bias:
                            for hf in range(2):
                                nc.tensor.matmul(
                                    pst[:, qi, hf, :],
                                    lhsT=b_s[:, q * CH:(q + 1) * CH],
                                    rhs=ones_s[:],
                                    start=False, stop=False,
                                    skip_group_check=True)

            def emit_step(t):
                g, hf = divmod(t, 2)
                psA, psB = ps_groups[g]
                B = state["B"]
                for j in range(KT):
                    for qi, q in ((0, 1), (1, 0)):
                        nc.tensor.matmul(
                            psA[:, qi, hf, :],
                            lhsT=wh_s[:, j, q * CH:(q + 1) * CH],
                            rhs=B[:, j, :],
                            start=False, stop=(j == KT - 1),
                            skip_group_check=True)
                if debug and t == 0:
                    dt_ = cpool.tile([P, NQ, N], F32)
                    for q in range(NQ):
                        nc.vector.tensor_copy(dt_[:, q, :], ps[:, q, hf, :])
                    nc.sync.dma_start(out=dbga[:], in_=dt_[:])
                    dtb = cpool.tile([P, KT, N], F32)
                    nc.vector.tensor_copy(dtb[:], state["B"][:])
                    nc.sync.dma_start(out=dbgb[:], in_=dtb[:])
                # f,i sigmoid on bank A; overlaps the g,o matmuls below
                sig = wpool.tile([P, 2, N], F32, name="sig", tag="sig")
                nc.scalar.activation(sig[:], psA[:, :, hf, :], AF.Sigmoid)
                c_new = wpool.tile([P, N], F32, name="c", tag="c")
                nc.vector.tensor_mul(out=c_new[:], in0=sig[:, 0, :],
                                     in1=state["c"][:])
                for j in range(KT):
                    for qi, q in ((0, 3), (1, 2)):
                        nc.tensor.matmul(
                            psB[:, qi, hf, :],
                            lhsT=wh_s[:, j, q * CH:(q + 1) * CH],
                            rhs=B[:, j, :],
                            start=False, stop=(j == KT - 1),
                            skip_group_check=True)
                if hf == 1:
                    del ps_groups[g]
                gg = wpool.tile([P, N], F32, name="gg", tag="gg")
                nc.scalar.activation(gg[:], psB[:, 0, hf, :], AF.Tanh)
                so = wpool.tile([P, N], F32, name="so", tag="so")
                nc.scalar.activation(so[:], psB[:, 1, hf, :], AF.Sigmoid)
                ig = wpool.tile([P, N], F32, name="ig", tag="ig")
                nc.vector.tensor_mul(out=ig[:], in0=sig[:, 1, :], in1=gg[:])
                nc.vector.tensor_add(out=c_new[:], in0=c_new[:], in1=ig[:])
                state["c"] = c_new
                tch = wpool.tile([P, N], F32, name="tch", tag="tch")
                nc.scalar.activation(tch[:], c_new[:], AF.Tanh)
                h_new = wpool.tile([P, N], BF16, name="hs", tag="hs")
                nc.vector.tensor_mul(out=h_new[:], in0=so[:], in1=tch[:])
                nc.scalar.dma_start(out=outh[t], in_=h_new[:])
                if t < n_steps - 1:
                    state["B"] = emit_exchange(h_new)

            for ph in range(n_steps + LOOKAHEAD):
                if ph < n_steps and ph % 2 == 0:
                    emit_u_group(ph // 2)
                if FILLER:
                    emit_filler()
                t = ph - LOOKAHEAD
                if t >= 0:
                    emit_step(t)
            if FILLER:
                sink = cpool.tile([P, 16], F32)
                nc.vector.tensor_copy(sink[:], ps_junk[:, 0:16])
                nc.sync.dma_start(out=sinko[:], in_=sink[:])

    nc.compile()
    return nc


def kernel(x, A, Wx, Wh, b):
    import ml_dtypes

    x = np.ascontiguousarray(np.asarray(x, dtype=np.float32))
    A = np.ascontiguousarray(np.asarray(A, dtype=np.float32))
    Wx = np.asarray(Wx, dtype=np.float32)
    Wh = np.asarray(Wh, dtype=np.float32)
    b = np.asarray(b, dtype=np.float32)

    with_bias = bool(np.any(b))
    n_steps = int(os.environ.get("KERNEL_STEPS", T))
    key = (with_bias, n_steps)
    if key not in _cached:
        _cached[key] = _build(with_bias, n_steps)
    nc = _cached[key]

    mmnp = ml_dtypes.bfloat16
    xT_np = np.ascontiguousarray(
        x.transpose(2, 1, 0).reshape(D, T * N).astype(mmnp))

    in_maps = []
    for k in range(NCORES):
        cols = np.concatenate([np.arange(q * H + k * CH, q * H + k * CH + CH)
                               for q in range(4)])
        m = {
            "xT": xT_np,
            "wx": np.ascontiguousarray(Wx[:, cols].astype(mmnp)),
            "wh": np.ascontiguousarray(Wh[:, cols].astype(mmnp)),
            "ach": np.ascontiguousarray(
                A[:, k * CH:(k + 1) * CH].transpose(1, 0, 2, 3).reshape(P, N * 100)),
        }
        if with_bias:
            m["bvec"] = np.ascontiguousarray(b[cols].reshape(1, GC).astype(mmnp))
            m["ones"] = np.ones((1, N), dtype=mmnp)
        in_maps.append(m)

    res = run_bass_kernel_spmd(nc, in_maps, core_ids=list(range(NCORES)))
    global last_result
    last_result = res

    final = np.empty((N, n_steps, H), dtype=np.float32)
    for k in range(NCORES):
        # outh: [T, 128, N] bf16, h^T chunks -> final[n, t, k*CH + c]
        final[:, :, k * CH:(k + 1) * CH] = (
            res.results[k]["outh"].astype(np.float32).transpose(2, 0, 1))
    return final
